# revision 17
# baseline (speedup 1.0000x reference)
"""Contrastive loss kernel for trn2 (8 NeuronCores, SPMD).

Computes (see reference): segment-mean embeddings from f1/csv_ids and
f2/wiki_ids, logits = csv_emb @ wiki_emb.T / T, masked log-softmax losses
along both axes, returns A0*axis0 + A1*axis1.

Strategy v3 (transposed tables + chunked ReduceScatter, host final scalars):
  - Host pre-scales f rows by the per-id count reciprocal, sorts rows by
    id and splits each id-window's rows evenly across the 8 cores
    (variable tiles per window, same schedule on every core).
  - Phase A: matmul(lhsT=f_tile[128n,128d], rhs=onehot[128n,128c])
    accumulates the [D, C]-layout mean table in PSUM; strip-half banks are
    evacuated bf16 to DRAM in half-major order so the wiki table's
    ReduceScatter + AllGather run chunked (2 halves) while the csv side
    still computes.
  - The csv strip from its ReduceScatter IS the logits lhsT (no divide, no
    transpose, no AllGather).
  - Logits: software-pipelined PE matmuls / ACT exp(10x-40) with accum_out
    row sums; column sums via ones-matmul PSUM accumulation.
  - Pair terms: pairs redistributed by csv strip on host; per c-window
    one-hot matmuls of dma_gathered wiki rows give M1/M0; DVE dots against
    the (r_c-weighted) csv strip produce u1/u0 partials.
  - Device ships rs_all/colsum/uacc; host does ln + reductions in f64.
"""
import sys
sys.path.insert(0, "/opt/trn_rl_repo")

import numpy as np
import ml_dtypes
from contextlib import ExitStack

import concourse.bass as bass
import concourse.tile as tile
from concourse import bacc, mybir
from concourse.bass_utils import run_bass_kernel_spmd

F32 = mybir.dt.float32
BF16 = mybir.dt.bfloat16
I16 = mybir.dt.int16
I32 = mybir.dt.int32
AF = mybir.ActivationFunctionType
OP = mybir.AluOpType

NCORES = 8
N, D = 131072, 128
C = W = 8192
B = N // NCORES
NWIN = 64                  # id windows of 128 per side
STRIP = C // NCORES        # table columns per core (logits rows)
SWIN = STRIP // 128        # strip windows (phase B / logits subs)
TEMP_INV = 10.0
SHIFT = 40.0
A0 = A1 = 0.5
PAD_REL = 999.0

# window processing order: strip-half h major, strip s, then the 4 windows
# of bank (s, h).  Bank proc index p == DRAM row p of the partial table;
# rows [0,8) are the h=0 halves of strips 0..7, rows [8,16) the h=1 halves.
WORDER = [8 * s + 4 * h + j
          for h in range(2) for s in range(8) for j in range(4)]


def _build(tiles_c, tiles_ww, ptw, stop="FULL"):
    """tiles_c/tiles_ww: per-window tile counts (in WORDER) for csv/wiki.
    ptw: pair tiles per strip window."""
    nt_c, nt_w = sum(tiles_c), sum(tiles_ww)
    ntB = SWIN * ptw
    nc = bacc.Bacc("TRN2", target_bir_lowering=False, debug=False,
                   num_devices=NCORES, num_swdge_queues=4)

    f1b = nc.dram_tensor("f1b", [128, nt_c * D], BF16, kind="ExternalInput")
    f2b = nc.dram_tensor("f2b", [128, nt_w * D], BF16, kind="ExternalInput")
    rel_c = nc.dram_tensor("rel_c", [128, nt_c], F32, kind="ExternalInput")
    rel_w = nc.dram_tensor("rel_w", [128, nt_w], F32, kind="ExternalInput")
    relB = nc.dram_tensor("relB", [128, ntB], F32, kind="ExternalInput")
    b0B = nc.dram_tensor("b0B", [128, ntB], F32, kind="ExternalInput")
    wgi = nc.dram_tensor("wgi", [128, ntB * 8], I16, kind="ExternalInput")
    rcs = nc.dram_tensor("rcs", [128, STRIP], BF16, kind="ExternalInput")

    rs_out = nc.dram_tensor("rs_out", [128, 64], F32, kind="ExternalOutput")
    cs_out = nc.dram_tensor("cs_out", [1, W], F32, kind="ExternalOutput")
    u_out = nc.dram_tensor("u_out", [128, 2 * SWIN], F32, kind="ExternalOutput")

    with tile.TileContext(nc) as tc, ExitStack() as ctx:
        const = ctx.enter_context(tc.tile_pool(name="const", bufs=1))
        persist = ctx.enter_context(tc.tile_pool(name="persist", bufs=1))
        dram = ctx.enter_context(tc.tile_pool(name="dram", bufs=1, space="DRAM"))

        # ---- constants
        iota_i = const.tile([128, 128], I32)
        nc.gpsimd.iota(iota_i[:], pattern=[[1, 128]], base=0, channel_multiplier=0)
        iota_bf = const.tile([128, 128], BF16)
        nc.vector.tensor_copy(out=iota_bf[:], in_=iota_i[:])
        pid_i = const.tile([128, 1], I32)
        nc.gpsimd.iota(pid_i[:], pattern=[[1, 1]], base=0, channel_multiplier=1)
        pid_f = const.tile([128, 1], F32)
        nc.vector.tensor_copy(out=pid_f[:], in_=pid_i[:])
        ident = const.tile([128, 128], BF16)
        nc.vector.tensor_scalar(out=ident[:], in0=iota_bf[:], scalar1=pid_f[:, 0:1],
                                scalar2=None, op0=OP.is_equal)
        ones_bf = const.tile([128, 1], BF16)
        nc.vector.memset(ones_bf[:], 1.0)
        bias_m40 = const.tile([128, 1], F32)
        nc.vector.memset(bias_m40[:], -SHIFT)

        # ---- small inputs
        rel_t = {}
        for key, src, ntk in (("w", rel_w, nt_w), ("c", rel_c, nt_c)):
            t = const.tile([128, ntk], F32, name=f"rel_{key}_t", tag=f"rel_{key}")
            nc.sync.dma_start(out=t[:], in_=src[:, :])
            rel_t[key] = t
        relB_t = const.tile([128, ntB], F32, name="relB_t", tag="relB")
        nc.sync.dma_start(out=relB_t[:], in_=relB[:, :])
        b0B_t = const.tile([128, ntB], F32, name="b0B_t", tag="b0B")
        nc.sync.dma_start(out=b0B_t[:], in_=b0B[:, :])
        wgi_t = const.tile([128, ntB * 8], I16, name="wgi_t", tag="wgi")
        nc.sync.dma_start(out=wgi_t[:], in_=wgi[:, :])
        rcs_t = const.tile([128, STRIP], BF16, name="rcs_t", tag="rcs")
        nc.sync.dma_start(out=rcs_t[:], in_=rcs[:, :])

        # ---- DRAM scratch
        part = {"w": dram.tile([16, 128 * 512], BF16, name="part_w"),
                "c": dram.tile([16, 128 * 512], BF16, name="part_c")}
        strip_d = {("w", 0): dram.tile([1, 128 * 512], BF16, name="strip_w0"),
                   ("w", 1): dram.tile([1, 128 * 512], BF16, name="strip_w1"),
                   ("c", 0): dram.tile([1, 128 * 512], BF16, name="strip_c0"),
                   ("c", 1): dram.tile([1, 128 * 512], BF16, name="strip_c1")}
        wiki_ag = [dram.tile([8, 128 * 512], BF16, name=f"wiki_ag{h}",
                             addr_space="Shared") for h in range(2)]
        wiki_rows = dram.tile([W, D], BF16, name="wiki_rows")

        def rs_half(side, h):
            nc.gpsimd.collective_compute(
                "ReduceScatter", OP.add, replica_groups=[list(range(NCORES))],
                ins=[part[side][8 * h:8 * (h + 1), :].opt()],
                outs=[strip_d[(side, h)].opt()])

        def ag_half(h):
            nc.gpsimd.collective_compute(
                "AllGather", OP.bypass, replica_groups=[list(range(NCORES))],
                ins=[strip_d[("w", h)].opt()], outs=[wiki_ag[h].opt()])

        # ================= phase A =================
        sides = [("w", f2b, tiles_ww, nt_w), ("c", f1b, tiles_c, nt_c)]
        with tc.tile_pool(name="ga", bufs=2) as gpool, \
             tc.tile_pool(name="wka", bufs=10) as wk, \
             tc.tile_pool(name="evac", bufs=4) as evp, \
             tc.tile_pool(name="psa", bufs=3, space="PSUM") as psa:
            for side, fparam, tiles, ntk in sides:
                pv = part[side].rearrange("g (p x) -> p g x", p=128)
                fall = gpool.tile([128, ntk, D], BF16, tag="fall")
                # chunked load so the first matmuls start early
                ldq = max(1, ntk // 4)
                for r0 in range(0, ntk, ldq):
                    r1 = min(ntk, r0 + ldq)
                    nc.sync.dma_start(out=fall[:, r0:r1, :],
                                      in_=fparam[:, r0 * D:r1 * D])
                t0 = 0
                grp_ps = None
                for pi, win in enumerate(WORDER):
                    k = pi % 4            # window slot within bank
                    proc = pi // 4        # bank proc index == DRAM row
                    ts, te = t0, t0 + tiles[pi]
                    t0 = te
                    for t in range(ts, te):
                        oh = wk.tile([128, 128], BF16, tag="oh")
                        nc.vector.tensor_scalar(
                            out=oh[:], in0=iota_bf[:],
                            scalar1=rel_t[side][:, t:t + 1],
                            scalar2=None, op0=OP.is_equal)
                        if k == 0 and t == ts:
                            grp_ps = psa.tile([128, 512], F32, tag="winps")
                        nc.tensor.matmul(grp_ps[:, k * 128:(k + 1) * 128],
                                         lhsT=fall[:, t, :], rhs=oh[:],
                                         start=(t == ts), stop=(t == te - 1))
                    if k == 3:
                        stg = evp.tile([128, 512], BF16, tag="stg")
                        nc.vector.tensor_copy(out=stg[:], in_=grp_ps[:])
                        nc.sync.dma_start(out=pv[:, proc, :], in_=stg[:])
                        if stop != "A" and side == "w":
                            if proc == 7:
                                rs_half("w", 0)
                                ag_half(0)
                            elif proc == 15:
                                rs_half("w", 1)
                                ag_half(1)
            if stop != "A":
                rs_half("c", 0)
                rs_half("c", 1)

        done = stop in ("A", "RS")

        # ================= tables to SBUF =================
        # wiki_T columns ordered (strip s, half h, x) == global id
        wiki_T = persist.tile([128, 8, 2, 512], BF16, tag="wiki_T")
        csv_lhsT = persist.tile([128, STRIP], BF16, tag="csv_lhsT")
        csv_w1 = persist.tile([128, STRIP], BF16, tag="csv_w1")

        # wiki_rows: transpose wiki_T tiles -> [W, D] row table in DRAM
        wrv = wiki_rows.rearrange("(t p) d -> p t d", p=128)
        with tc.tile_pool(name="trp", bufs=2, space="PSUM") as pst, \
             tc.tile_pool(name="trs", bufs=2) as trs:
            for h in range(2 if not done else 0):
                agv = wiki_ag[h].rearrange("s (p x) -> p s x", p=128)
                nc.sync.dma_start(out=wiki_T[:, :, h, :], in_=agv[:])
                for s in range(8):
                    wr_stg = trs.tile([128, 4, 128], BF16, tag="wrstg")
                    for q in range(4):
                        tp = pst.tile([128, 128], BF16, tag="trps")
                        nc.tensor.transpose(
                            tp[:], wiki_T[:, s, h, q * 128:(q + 1) * 128],
                            ident[:])
                        nc.vector.tensor_copy(out=wr_stg[:, q, :], in_=tp[:])
                    # global 128-row block t = s*8 + h*4 + q
                    nc.sync.dma_start(out=wrv[:, s * 8 + h * 4:s * 8 + h * 4 + 4, :],
                                      in_=wr_stg[:])

        if not done:
            for h in range(2):
                csv_v = strip_d[("c", h)].rearrange("a (p x) -> p (a x)", p=128)
                nc.sync.dma_start(out=csv_lhsT[:, h * 512:(h + 1) * 512],
                                  in_=csv_v[:])

        if stop == "TAB" and not done:
            done = True

        # ================= logits + pair terms =================
        rs_all = persist.tile([128, 64], F32, tag="rs_all")
        colsum = persist.tile([1, W], F32, tag="colsum")
        uacc = persist.tile([128, 2 * SWIN], F32, tag="uacc")

        with tc.tile_pool(name="gb", bufs=1) as gb, \
             tc.tile_pool(name="wkl", bufs=12) as wk, \
             tc.tile_pool(name="scrp", bufs=2) as scrp, \
             tc.tile_pool(name="psl", bufs=2, space="PSUM") as psl, \
             tc.tile_pool(name="psc", bufs=1, space="PSUM") as psc, \
             tc.tile_pool(name="psm", bufs=2, space="PSUM") as psm:
            # pair-term one-hots: built during the collective wall (only
            # need relB/b0B), stored until the M matmuls consume them
            pbo = gb.tile([128, ntB, 2, 128], BF16, tag="pbo")
            for j in range(ntB if not done else 0):
                nc.vector.tensor_scalar(
                    out=pbo[:, j, 0, :], in0=iota_bf[:],
                    scalar1=relB_t[:, j:j + 1], scalar2=None, op0=OP.is_equal)
                nc.vector.tensor_scalar(
                    out=pbo[:, j, 1, :], in0=iota_bf[:],
                    scalar1=relB_t[:, j:j + 1], scalar2=b0B_t[:, j:j + 1],
                    op0=OP.is_equal, op1=OP.mult)
            if not done:
                nc.vector.tensor_tensor(out=csv_w1[:], in0=csv_lhsT[:],
                                        in1=rcs_t[:], op=OP.mult)
            wg = []
            for v in range(SWIN if not done else 0):
                g = gb.tile([128, ptw, D], BF16, name=f"wg{v}", tag=f"wg{v}")
                nc.gpsimd.dma_gather(
                    out_ap=g[:], in_ap=wiki_rows[:, :],
                    idxs_ap=wgi_t[:, v * ptw * 8:(v + 1) * ptw * 8],
                    num_idxs=ptw * 128, num_idxs_reg=ptw * 128, elem_size=D,
                    single_packet=False, queue_num=v % 4)
                wg.append(g)

            exq = []   # deferred per-chunk colsum + pair-window work

            def tail_work(k):
                ex8 = exq[k]
                cs_a = psc.tile([1, 512], F32, tag="cs_a")
                cs_b = psc.tile([1, 512], F32, tag="cs_b")
                for s in range(8):
                    nc.tensor.matmul(cs_a[:], lhsT=ones_bf[:],
                                     rhs=ex8[s][:, 0:512],
                                     start=(s == 0), stop=(s == 7))
                    nc.tensor.matmul(cs_b[:], lhsT=ones_bf[:],
                                     rhs=ex8[s][:, 512:1024],
                                     start=(s == 0), stop=(s == 7))
                nc.vector.tensor_copy(
                    out=colsum[0:1, k * 1024:k * 1024 + 512], in_=cs_a[:])
                nc.vector.tensor_copy(
                    out=colsum[0:1, k * 1024 + 512:(k + 1) * 1024], in_=cs_b[:])
                # pair window k
                mps = psm.tile([128, 256], F32, tag="mps")
                for j in range(ptw):
                    col = k * ptw + j
                    nc.tensor.matmul(mps[:, 0:128], lhsT=wg[k][:, j, :],
                                     rhs=pbo[:, col, 0, :], start=(j == 0),
                                     stop=(j == ptw - 1))
                    nc.tensor.matmul(mps[:, 128:256], lhsT=wg[k][:, j, :],
                                     rhs=pbo[:, col, 1, :], start=(j == 0),
                                     stop=(j == ptw - 1))
                scr = scrp.tile([128, 128], F32, tag="scr")
                nc.vector.scalar_tensor_tensor(
                    out=scr[:], in0=mps[:, 0:128], scalar=1.0, op0=OP.mult,
                    in1=csv_w1[:, k * 128:(k + 1) * 128], op1=OP.mult,
                    accum_out=uacc[:, k:k + 1])
                scr2 = scrp.tile([128, 128], F32, tag="scr2")
                nc.vector.scalar_tensor_tensor(
                    out=scr2[:], in0=mps[:, 128:256], scalar=1.0, op0=OP.mult,
                    in1=csv_lhsT[:, k * 128:(k + 1) * 128], op1=OP.mult,
                    accum_out=uacc[:, SWIN + k:SWIN + k + 1])

            for k in range(8 if not done else 0):
                ex8 = []
                for s in range(8):
                    lp = psl.tile([128, 1024], F32, tag="lp")
                    nc.tensor.matmul(
                        lp[:, 0:512], lhsT=csv_lhsT[:, s * 128:(s + 1) * 128],
                        rhs=wiki_T[:, k, 0, :], start=True, stop=True)
                    nc.tensor.matmul(
                        lp[:, 512:1024], lhsT=csv_lhsT[:, s * 128:(s + 1) * 128],
                        rhs=wiki_T[:, k, 1, :], start=True, stop=True)
                    ex = wk.tile([128, 1024], BF16, tag="ex")
                    nc.scalar.activation(
                        out=ex[:], in_=lp[:], func=AF.Exp, scale=TEMP_INV,
                        bias=bias_m40[:, 0:1],
                        accum_out=rs_all[:, s * 8 + k:s * 8 + k + 1])
                    ex8.append(ex)
                exq.append(ex8)
                if k >= 1:
                    tail_work(k - 1)
            if not done:
                tail_work(7)

        if not done:
            nc.sync.dma_start(out=rs_out[:, :], in_=rs_all[:])
            nc.sync.dma_start(out=cs_out[:, :], in_=colsum[:])
            nc.sync.dma_start(out=u_out[:, :], in_=uacc[:])
        else:
            zz = persist.tile([128, 64], F32, tag="zz")
            nc.vector.memset(zz[:], 1.0)
            nc.sync.dma_start(out=rs_out[:, :], in_=zz[:])
            zc = persist.tile([1, W], F32, tag="zc")
            nc.vector.memset(zc[:], 1.0)
            nc.sync.dma_start(out=cs_out[:, :], in_=zc[:])
            zu = persist.tile([128, 2 * SWIN], F32, tag="zu")
            nc.vector.memset(zu[:], 0.0)
            nc.sync.dma_start(out=u_out[:, :], in_=zu[:])

    nc.finalize()
    return nc


# ------------------------------------------------------------------- host ---


def _wrap16(a):
    """[num] int16 -> [128, num//16] gather-index layout (16-wrap, 8x repl)."""
    return np.ascontiguousarray(np.tile(a.reshape(-1, 16).T, (8, 1)))


def _col128(a, nt):
    """[nt*128] -> [128, nt] tile-column layout."""
    return np.ascontiguousarray(a.reshape(nt, 128).T)


_CACHE = {}


def _run(inputs, trace=False, tmpdir=None):
    f1 = np.asarray(inputs["f1"], np.float32)
    f2 = np.asarray(inputs["f2"], np.float32)
    ci = np.asarray(inputs["csv_ids"]).astype(np.int64)
    wi = np.asarray(inputs["wiki_ids"]).astype(np.int64)

    cnt_c = np.bincount(ci, minlength=C).astype(np.float64)
    cnt_w = np.bincount(wi, minlength=W).astype(np.float64)
    r_c = (1.0 / np.maximum(cnt_c, 1.0)).astype(np.float32)
    r_w = (1.0 / np.maximum(cnt_w, 1.0)).astype(np.float32)
    g_c = (cnt_c > 0).astype(np.float64)
    g_w = (cnt_w > 0).astype(np.float64)

    # per-side per-window tile counts (in WORDER)
    def win_tiles(ids):
        gcnt = np.bincount(ids >> 7, minlength=NWIN)
        share = -(-gcnt // NCORES)
        return tuple(int(-(-share[w] // 128)) for w in WORDER)

    tiles_c = win_tiles(ci)
    tiles_ww = win_tiles(wi)

    # phase B pair windows (global, by csv id)
    orderB = np.argsort(ci, kind="stable")
    csB = ci[orderB]
    wsB = wi[orderB]
    b0all = r_w[wsB].astype(np.float32)
    startsB = np.searchsorted(csB, np.arange(NWIN) * 128)
    endsB = np.searchsorted(csB, np.arange(1, NWIN + 1) * 128)
    ptw = max(1, int(max(-(-(endsB - startsB) // 128))))
    ntB = SWIN * ptw

    import os as _os
    stop = _os.environ.get("KSTOP", "FULL")
    key = (tiles_c, tiles_ww, ptw, stop)
    if key not in _CACHE:
        _CACHE[key] = _build(tiles_c, tiles_ww, ptw, stop=stop)
    nc = _CACHE[key]

    def side_prep(f, ids, recip, tiles):
        nt = sum(tiles)
        fs = (f * recip[ids][:, None]).astype(ml_dtypes.bfloat16)
        fb_all = [np.zeros((nt * 128, D), ml_dtypes.bfloat16)
                  for _ in range(NCORES)]
        rel_all = [np.full(nt * 128, PAD_REL, np.float32)
                   for _ in range(NCORES)]
        order = np.argsort(ids, kind="stable")
        srt = ids[order]
        ws = np.searchsorted(srt, np.arange(NWIN) * 128)
        we = np.searchsorted(srt, np.arange(1, NWIN + 1) * 128)
        base = 0
        for pi, w in enumerate(WORDER):
            rows = order[ws[w]:we[w]]
            rids = srt[ws[w]:we[w]]
            nw = len(rows)
            share = -(-nw // NCORES)
            for i in range(NCORES):
                sl = slice(i * share, min((i + 1) * share, nw))
                cnt = max(0, sl.stop - sl.start)
                if cnt:
                    fb_all[i][base:base + cnt] = fs[rows[sl]]
                    rel_all[i][base:base + cnt] = (
                        rids[sl] - w * 128).astype(np.float32)
            base += tiles[pi] * 128
        outs = []
        for i in range(NCORES):
            fbp = np.ascontiguousarray(
                fb_all[i].reshape(nt, 128, D).transpose(1, 0, 2)
                .reshape(128, nt * D))
            outs.append((fbp, _col128(rel_all[i], nt)))
        return outs

    prep_c = side_prep(f1, ci, r_c, tiles_c)
    prep_w = side_prep(f2, wi, r_w, tiles_ww)

    in_maps = []
    for i in range(NCORES):
        relBp = np.full(ntB * 128, PAD_REL, np.float32)
        b0Bp = np.zeros(ntB * 128, np.float32)
        wgp = np.zeros(ntB * 128, np.int16)
        for v in range(SWIN):
            gwv = i * SWIN + v
            s, e = startsB[gwv], endsB[gwv]
            cnt = e - s
            base = v * ptw * 128
            relBp[base:base + cnt] = (csB[s:e] - gwv * 128).astype(np.float32)
            b0Bp[base:base + cnt] = b0all[s:e]
            wgp[base:base + cnt] = wsB[s:e].astype(np.int16)
        wgi_arr = np.concatenate(
            [_wrap16(wgp[v * ptw * 128:(v + 1) * ptw * 128])
             for v in range(SWIN)], axis=1)
        rcs_arr = np.ascontiguousarray(np.broadcast_to(
            r_c[i * STRIP:(i + 1) * STRIP][None, :], (128, STRIP))
        ).astype(ml_dtypes.bfloat16)
        in_maps.append({
            "f1b": prep_c[i][0], "f2b": prep_w[i][0],
            "rel_c": prep_c[i][1], "rel_w": prep_w[i][1],
            "relB": _col128(relBp, ntB), "b0B": _col128(b0Bp, ntB),
            "wgi": wgi_arr, "rcs": rcs_arr,
        })

    res = run_bass_kernel_spmd(nc, in_maps, core_ids=list(range(NCORES)),
                               trace=trace, tmpdir=tmpdir)

    # ---- host combine (f64)
    u1 = u0 = 0.0
    v1 = 0.0
    cs_sum = np.zeros(W, np.float64)
    for i in range(NCORES):
        r = res.results[i]
        ua = np.asarray(r["u_out"], np.float64)
        u1 += ua[:, 0:SWIN].sum()
        u0 += ua[:, SWIN:2 * SWIN].sum()
        rs = np.asarray(r["rs_out"], np.float64)     # [128, 64], col = s*8+k
        rowsum = rs.reshape(128, 8, 8).sum(axis=2)   # [p, s]
        gs = g_c[i * STRIP:(i + 1) * STRIP].reshape(SWIN, 128).T
        v1 += (np.log(np.maximum(rowsum, 1e-300)) * gs).sum()
        cs_sum += np.asarray(r["cs_out"], np.float64)[0]
    v0 = (np.log(np.maximum(cs_sum, 1e-300)) * g_w).sum()
    G1 = g_c.sum()
    G0 = g_w.sum()
    ax1 = -(TEMP_INV * u1 - (v1 + SHIFT * G1)) / C
    ax0 = -(TEMP_INV * u0 - (v0 + SHIFT * G0)) / W
    loss = A0 * ax0 + A1 * ax1
    return np.float32(loss), res


def kernel(**inputs) -> np.ndarray:
    out, _ = _run(inputs)
    return out
